# revision 1
# baseline (speedup 1.0000x reference)
"""MoE FFN (8 experts, top-2, GLU) on 8 Trainium2 NeuronCores.

Strategy
--------
Phase 1 (on-device, data-parallel over tokens): each core computes router
logits for its 512-token shard, softmax-free top-2 gate weights
c[t, e] = z_e / (z_1 + z_2) for the two largest z (z = exp(logit - max)),
zero elsewhere.  Exactly matches softmax + top-k + L1-normalize.

Host dispatch (data movement only): for each expert, gather the columns of
x^T for its routed tokens into a fixed-capacity buffer.

Phase 2 (on-device, expert-parallel): core e computes the GLU FFN of expert
e over its gathered tokens with full-rate float32r matmuls:
    h = silu(w1t^T xg) * (v1t^T xg)   [F, C]
    y = h^T w2                        [C, H]
    y *= c                            (per-token gate weight)

Host combine (data movement only): out[idx_e] += y_e.

Weights are shipped pre-tiled ([FO,128,HO,128] etc.) so every weight DMA is
a single large contiguous descriptor per partition, and streamed in
half-tiles so the PE never waits on a full weight block.  x chunks, weight
tiles and outputs are spread over the SWDGE/HWDGE queues so the serial DMA
engine feeds the PE in dependency order.

The expert phase computes only the exact active token count CA (rounded to
8 for fp32r free-dim alignment) while buffers stay at capacity C, trimming
dead matmul rows; the h tail [CA:ceil128(CA)] is zeroed so padded psum
columns stay finite and gate to zero.

Measured (seed-0 inputs, 8 cores): relative error 2.77e-4 vs the fp32
reference; timeline-sim exec time ~208 us total (router 13.9 us + expert
~194 us at CA=1072; expert PE-busy ~178 us ~= the fp32r roofline).
"""

import numpy as np

import concourse.bacc as bacc
import concourse.mybir as mybir
import concourse.tile as tile
from concourse.bass_utils import run_bass_kernel_spmd

P = 128
E = 8
H = 1024
F = 2048
T = 4096
NCORES = 8
TSH = T // NCORES  # tokens per core in router phase
HO = H // P  # 8
FO = F // P  # 16
F32 = mybir.dt.float32
F32R = mybir.dt.float32r

_NC_CACHE = {}
_W_CACHE = {}
_USE_SILU_ACT = True


def _token_chunks(C):
    """Split C into free-dim chunks <= 512 (>= 256 when C >= 512)."""
    assert C >= 1
    chunks = []
    t0 = 0
    while t0 < C:
        rem = C - t0
        if rem >= 768:
            tl = 512
        elif rem > 512:
            tl = rem - 256  # leaves a final 256 chunk; both >= 256
        else:
            tl = rem
        chunks.append((t0, tl))
        t0 += tl
    return chunks


def _build_router():
    nc = bacc.Bacc("TRN2", target_bir_lowering=False, debug=False,
                   enable_partition_id=False)
    xT = nc.dram_tensor("xT", [H, TSH], F32, kind="ExternalInput")
    rwT = nc.dram_tensor("rwT", [H, E], F32, kind="ExternalInput")
    c_out = nc.dram_tensor("c", [TSH, E], F32, kind="ExternalOutput")
    with tile.TileContext(nc) as tc:
        with tc.tile_pool(name="xp", bufs=1) as xp, \
             tc.tile_pool(name="wp", bufs=1) as wp, \
             tc.tile_pool(name="sp", bufs=4) as sp, \
             tc.tile_pool(name="ps", bufs=4, space="PSUM") as ps:
            rw = wp.tile([P, HO, E], F32)
            nc.sync.dma_start(rw[:], rwT.ap().rearrange("(ho p) e -> p ho e", p=P))
            HH = HO // 2
            xts = []
            for tt in range(TSH // P):
                halves = []
                for hf in range(2):
                    xt = xp.tile([P, HH, P], F32, tag=f"xt{tt}_{hf}",
                                 name=f"xt{tt}_{hf}")
                    nc.sync.dma_start(
                        xt[:],
                        xT.ap()[hf * HH * P:(hf + 1) * HH * P,
                                tt * P:(tt + 1) * P].rearrange(
                            "(ho p) t -> p ho t", p=P))
                    halves.append(xt)
                xts.append(halves)
            pls = [ps.tile([P, E], F32, tag="pl", name=f"pl{tt}")
                   for tt in range(TSH // P)]
            for tt in range(TSH // P):
                for ho in range(HO):
                    nc.tensor.matmul(pls[tt][:],
                                     xts[tt][ho // HH][:, ho % HH, :],
                                     rw[:, ho, :],
                                     start=(ho == 0), stop=(ho == HO - 1))
            for tt in range(TSH // P):
                pl = pls[tt]
                nmax = sp.tile([P, 1], F32, tag="nmax")
                nc.vector.tensor_reduce(nmax[:], pl[:], axis=mybir.AxisListType.X,
                                        op=mybir.AluOpType.max, negate=True)
                z = sp.tile([P, E], F32, tag="z")
                nc.scalar.activation(z[:], pl[:], mybir.ActivationFunctionType.Exp,
                                     bias=nmax[:, 0:1], scale=1.0)
                m8 = sp.tile([P, 8], F32, tag="m8")
                nc.vector.max(m8[:], z[:])
                s2 = sp.tile([P, 1], F32, tag="s2")
                nc.vector.tensor_reduce(s2[:], m8[:, 0:2], axis=mybir.AxisListType.X,
                                        op=mybir.AluOpType.add)
                rec = sp.tile([P, 1], F32, tag="rec")
                nc.vector.reciprocal(rec[:], s2[:])
                cm = sp.tile([P, E], F32, tag="cm")
                nc.vector.tensor_scalar(cm[:], z[:], m8[:, 1:2], None,
                                        op0=mybir.AluOpType.is_ge)
                cg = sp.tile([P, E], F32, tag="cg")
                nc.vector.tensor_mul(cg[:], z[:], cm[:])
                nc.vector.tensor_scalar_mul(cg[:], cg[:], rec[:, 0:1])
                nc.sync.dma_start(c_out.ap()[tt * P:(tt + 1) * P, :], cg[:])
    nc.compile()
    return nc


def _build_expert(C, CA=None):
    if CA is None:
        CA = C
    assert 1 <= CA <= C
    CA = min(C, ((CA + 7) // 8) * 8)  # fp32r free-dim alignment
    TO = (CA + P - 1) // P  # active 128-token blocks
    CH = TO * P             # h width (>= CA, <= C)
    nc = bacc.Bacc("TRN2", target_bir_lowering=False, debug=False,
                   enable_partition_id=False)
    xgT = nc.dram_tensor("xgT", [H, C], F32R, kind="ExternalInput")
    cgt = nc.dram_tensor("cgt", [P, TO], F32, kind="ExternalInput")
    w1t = nc.dram_tensor("w1t", [FO, P, HO, P], F32R, kind="ExternalInput")
    v1t = nc.dram_tensor("v1t", [FO, P, HO, P], F32R, kind="ExternalInput")
    w2t = nc.dram_tensor("w2t", [2, P, FO, H // 2], F32R, kind="ExternalInput")
    y = nc.dram_tensor("y", [C, H], F32, kind="ExternalOutput")
    chunks = _token_chunks(CA)
    with tile.TileContext(nc) as tc:
        with tc.tile_pool(name="xp", bufs=1) as xp, \
             tc.tile_pool(name="hp", bufs=1) as hp, \
             tc.tile_pool(name="wp", bufs=2) as wp, \
             tc.tile_pool(name="w2p", bufs=28) as w2p, \
             tc.tile_pool(name="cp", bufs=1) as cp, \
             tc.tile_pool(name="yp", bufs=6) as yp, \
             tc.tile_pool(name="ps", bufs=3, space="PSUM") as ps, \
             tc.tile_pool(name="psb", bufs=2, space="PSUM") as psb:
            HH = HO // 2

            def load_wv_half(fo, half):
                w1s = wp.tile([P, HH, P], F32R, tag=f"w1s{half}",
                              name=f"w1s{fo}_{half}")
                nc.sync.dma_start(
                    w1s[:], w1t.ap()[fo, :, half * HH:(half + 1) * HH, :])
                v1s = wp.tile([P, HH, P], F32R, tag=f"v1s{half}",
                              name=f"v1s{fo}_{half}")
                nc.sync.dma_start(
                    v1s[:], v1t.ap()[fo, :, half * HH:(half + 1) * HH, :])
                return (w1s, v1s)

            def load_wv(fo):
                return [load_wv_half(fo, 0), load_wv_half(fo, 1)]

            def wsl(halves, mat, ho):
                return halves[ho // HH][mat][:, ho % HH, :]

            xgs = []
            for ho in range(HO):
                xgc = xp.tile([P, CA], F32R, tag=f"xg{ho}", name=f"xg{ho}")
                dma = nc.sync.dma_start if ho == 0 else nc.gpsimd.dma_start
                dma(xgc[:], xgT.ap()[ho * P:(ho + 1) * P, :CA])
                xgs.append(xgc)
            h = hp.tile([P, FO, CH], F32R)
            if CA < CH:
                nc.vector.memset(h[:, :, CA:CH].bitcast(mybir.dt.uint32), 0)
            cgs = cp.tile([P, TO], F32)
            nc.gpsimd.dma_start(cgs[:], cgt.ap())


            def glu_tail(fo, t0, tl, p1, p2):
                hs = h[:, fo, t0:t0 + tl]
                if _USE_SILU_ACT:
                    nc.scalar.activation(hs, p1,
                                         mybir.ActivationFunctionType.Silu)
                    nc.vector.tensor_mul(hs, hs, p2)
                else:
                    # silu(a)*b = a*sigmoid(a)*b (sim fallback)
                    sg = yp.tile([P, 512], F32, tag="sg", name="sg")[:, :tl]
                    nc.scalar.activation(sg, p1,
                                         mybir.ActivationFunctionType.Sigmoid)
                    nc.vector.tensor_mul(hs, p1, sg)
                    nc.vector.tensor_mul(hs, hs, p2)

            if len(chunks) > 3:
                # psum-budget fallback: chunk-serial accumulation
                for fo in range(FO):
                    wv = load_wv(fo)
                    for ci, (t0, tl) in enumerate(chunks):
                        p1 = ps.tile([P, 512], F32, tag="ps1", name="p1")[:, :tl]
                        p2 = ps.tile([P, 512], F32, tag="ps2", name="p2")[:, :tl]
                        for ho in range(HO):
                            st, sp_ = (ho == 0), (ho == HO - 1)
                            nc.tensor.matmul(p1, wsl(wv, 0, ho),
                                             xgs[ho][:, t0:t0 + tl],
                                             start=st, stop=sp_)
                            nc.tensor.matmul(p2, wsl(wv, 1, ho),
                                             xgs[ho][:, t0:t0 + tl],
                                             start=st, stop=sp_)
                        glu_tail(fo, t0, tl, p1, p2)
            else:
                    # Phase A: h = silu(w1t^T @ xg) * (v1t^T @ xg), laid out [f, t].
                # Prologue runs fo=0 (all chunks) plus fo=1's first chunk with
                # ho-outer order so the PE chases the streaming xg chunks.
                wv0 = load_wv(0)
                wv1 = load_wv(1)
                t00, tl0 = chunks[0]
                ps1s = [ps.tile([P, 512], F32, tag="ps1", name=f"ps1_{i}")[:, :tl]
                        for i, (t0, tl) in enumerate(chunks)]
                ps2s = [ps.tile([P, 512], F32, tag="ps2", name=f"ps2_{i}")[:, :tl]
                        for i, (t0, tl) in enumerate(chunks)]
                pre1 = psb.tile([P, 512], F32, tag="psy", name="pre1")[:, :tl0]
                pre2 = psb.tile([P, 512], F32, tag="psy", name="pre2")[:, :tl0]
                for ho in range(HO):
                    st, sp_ = (ho == 0), (ho == HO - 1)
                    for i, (t0, tl) in enumerate(chunks):
                        nc.tensor.matmul(ps1s[i], wsl(wv0, 0, ho),
                                         xgs[ho][:, t0:t0 + tl], start=st, stop=sp_)
                        nc.tensor.matmul(ps2s[i], wsl(wv0, 1, ho),
                                         xgs[ho][:, t0:t0 + tl], start=st, stop=sp_)
                    nc.tensor.matmul(pre1, wsl(wv1, 0, ho),
                                     xgs[ho][:, t00:t00 + tl0], start=st, stop=sp_)
                    nc.tensor.matmul(pre2, wsl(wv1, 1, ho),
                                     xgs[ho][:, t00:t00 + tl0], start=st, stop=sp_)
                for i, (t0, tl) in enumerate(chunks):
                    glu_tail(0, t0, tl, ps1s[i], ps2s[i])
                glu_tail(1, t00, tl0, pre1, pre2)

                for fo in range(1, FO):
                    if fo == 1:
                        wv = wv1
                        fo_chunks = chunks[1:]
                    else:
                        wv = load_wv(fo)
                        fo_chunks = chunks
                    ps1s = [ps.tile([P, 512], F32, tag="ps1", name=f"ps1_{i}")[:, :tl]
                            for i, (t0, tl) in enumerate(fo_chunks)]
                    ps2s = [ps.tile([P, 512], F32, tag="ps2", name=f"ps2_{i}")[:, :tl]
                            for i, (t0, tl) in enumerate(fo_chunks)]
                    for ho in range(HO):
                        st, sp_ = (ho == 0), (ho == HO - 1)
                        for i, (t0, tl) in enumerate(fo_chunks):
                            nc.tensor.matmul(ps1s[i], wsl(wv, 0, ho),
                                             xgs[ho][:, t0:t0 + tl],
                                             start=st, stop=sp_)
                            nc.tensor.matmul(ps2s[i], wsl(wv, 1, ho),
                                             xgs[ho][:, t0:t0 + tl],
                                             start=st, stop=sp_)
                    for i, (t0, tl) in enumerate(fo_chunks):
                        glu_tail(fo, t0, tl, ps1s[i], ps2s[i])

            # Phase B: y[t, :] = (h^T @ w2) * c[t]
            for hh in range(2):
                w2tiles = []
                for fo in range(FO):
                    w2s = w2p.tile([P, H // 2], F32R, tag="w2s",
                                   name=f"w2s_{hh}_{fo}")
                    nc.sync.dma_start(w2s[:], w2t.ap()[hh, :, fo, :])
                    w2tiles.append(w2s)
                for to in range(TO):
                    last = (hh == 1 and to == TO - 1)
                    # Final block: two half-width groups so the first half's
                    # gate-mul + store hide under the second half's matmuls.
                    parts = ((0, 256), (256, 256)) if last else ((0, 512),)
                    for (h0, hl) in parts:
                        psy = psb.tile([P, 512], F32, tag="psy",
                                       name="psy")[:, :hl]
                        for fo in range(FO):
                            nc.tensor.matmul(psy,
                                             h[:, fo, to * P:(to + 1) * P],
                                             w2tiles[fo][:, h0:h0 + hl],
                                             start=(fo == 0),
                                             stop=(fo == FO - 1))
                        yt = yp.tile([P, 512], F32, tag="yt", name="yt")[:, :hl]
                        nc.vector.tensor_scalar_mul(yt, psy, cgs[:, to:to + 1])
                        nc.sync.dma_start(
                            y.ap()[to * P:(to + 1) * P,
                                   hh * 512 + h0:hh * 512 + h0 + hl], yt)
    nc.compile()
    return nc


def _get_nc(key, builder):
    if key not in _NC_CACHE:
        _NC_CACHE[key] = builder()
    return _NC_CACHE[key]


def _tile_weights(w1, v1, w2):
    """Pre-tile the expert weights for large-descriptor DMA.

    w1t/v1t: [E, FO, 128(h), HO, 128(f)]  (lhsT tiles of [H,F] transposed mats)
    w2t:     [E, 2, 128(f), FO, 512(h)]
    """
    key = (w1.shape, w1.dtype.str, w1[0, 0, :4].tobytes(), w2[0, 0, :4].tobytes(),
           v1[0, 0, :4].tobytes(), float(w1[-1, -1, -1]), float(w2[-1, -1, -1]))
    if key in _W_CACHE:
        return _W_CACHE[key]
    # w1[e] is [F, H]; lhsT tile (fo): [p_h, ho, q_f] = w1[e][fo*128+q, ho*128+p]
    w1t = np.ascontiguousarray(
        w1.reshape(E, FO, P, HO, P).transpose(0, 1, 4, 3, 2))
    v1t = np.ascontiguousarray(
        v1.reshape(E, FO, P, HO, P).transpose(0, 1, 4, 3, 2))
    # w2[e] is [F, H]; tile (hh): [p_f, fo, j_h] = w2[e][fo*128+p, hh*512+j]
    w2t = np.ascontiguousarray(
        w2.reshape(E, FO, P, 2, H // 2).transpose(0, 3, 2, 1, 4))
    _W_CACHE.clear()
    _W_CACHE[key] = (w1t, v1t, w2t)
    return w1t, v1t, w2t


def kernel(x, router_w, w1, v1, w2):
    x = np.asarray(x, dtype=np.float32)
    router_w = np.asarray(router_w, dtype=np.float32)
    w1 = np.asarray(w1, dtype=np.float32)
    v1 = np.asarray(v1, dtype=np.float32)
    w2 = np.asarray(w2, dtype=np.float32)

    xf = x.reshape(T, H)
    xT = np.ascontiguousarray(xf.T)  # [H, T]
    rwT = np.ascontiguousarray(router_w.T)  # [H, E]

    # ---- Phase 1: router on device (data-parallel over tokens) ----
    nc1 = _get_nc("router", _build_router)
    in1 = [{"xT": np.ascontiguousarray(xT[:, i * TSH:(i + 1) * TSH]), "rwT": rwT}
           for i in range(NCORES)]
    r1 = run_bass_kernel_spmd(nc1, in1, core_ids=list(range(NCORES)))
    c = np.concatenate([r["c"] for r in r1.results], axis=0)  # [T, E]

    # ---- Host dispatch: gather tokens per expert (data movement only) ----
    idxs = [np.flatnonzero(c[:, e] != 0.0) for e in range(E)]
    maxc = max(len(ix) for ix in idxs)
    # Per-launch capacity; >1280 tokens per expert (never happens with
    # balanced routing) is handled by running the same NEFF multiple times.
    C = max(1152, min(1280, ((maxc + 127) // 128) * 128))
    nseg = (maxc + C - 1) // C

    w1t, v1t, w2t = _tile_weights(w1, v1, w2)

    out = np.zeros((T, H), np.float32)
    for seg in range(nseg):
        segixs = [idxs[e][seg * C:(seg + 1) * C] for e in range(E)]
        CA = max(1, max(len(ix) for ix in segixs))  # exact active count
        TO = (CA + P - 1) // P
        nc2 = _get_nc(("expert", C, CA), lambda: _build_expert(C, CA))
        in2 = []
        for e in range(E):
            ix = segixs[e]
            xgT = np.zeros((H, C), np.float32)
            xgT[:, :len(ix)] = xT[:, ix]
            cge = np.zeros((TO * P,), np.float32)
            cge[:len(ix)] = c[ix, e]
            cgt = np.ascontiguousarray(cge.reshape(TO, P).T)  # [P, TO]
            in2.append({"xgT": xgT, "cgt": cgt,
                        "w1t": w1t[e], "v1t": v1t[e], "w2t": w2t[e]})
        r2 = run_bass_kernel_spmd(nc2, in2, core_ids=list(range(NCORES)))
        # ---- Host combine: scatter-add per-expert outputs ----
        for e in range(E):
            ix = segixs[e]
            out[ix] += r2.results[e]["y"][:len(ix)]
    return out.reshape(x.shape)



# revision 3
# speedup vs baseline: 1.0088x; 1.0088x over previous
"""MoE FFN (8 experts, top-2, GLU) on 8 Trainium2 NeuronCores.

Strategy
--------
Phase 1 (on-device, data-parallel over tokens): each core computes router
logits in fp32 for its 512-token shard, then top-2 gate weights
c[t, e] = z_e / (z_1 + z_2) with z = exp(logit) for the two largest
(identical to softmax + top-k + L1-normalize; no max-subtraction is
needed because |logit| <~ 5 for these scales).  Router stays fp32 so the
top-2 SELECTION matches the fp32 reference exactly (bf16 logits would
flip near-tie tokens and blow the error budget).

Host dispatch (data movement only): for each expert, gather the columns of
bf16 x^T for its routed tokens into a fixed-capacity buffer.

Phase 2 (on-device, expert-parallel, bf16 operands / fp32 accumulate):
core e computes the GLU FFN of expert e over its CA gathered tokens:
    h = silu(w1t^T xg) * (v1t^T xg)      [F, CA]   (h stored bf16)
    yT[hb] = sum_fo w2t[fo,hb]^T h[fo]   [H, CA]   (output transposed:
             H on partitions, tokens on the free dim -> no token-padding
             to 128 in the second GEMM)
    yT *= cb                             (gate broadcast along partitions)
bf16 matmuls run at the same 1 cycle/row as fp32r but halve every DMA
byte, which shrinks the pipeline head and keeps the PE continuously fed.
w1/v1 ship interleaved as one DMA per fo-block; x chunks go out on the
gpsimd (SWDGE) queue so the SP sequencer only carries the critical path.

Host combine (data movement only): out[idx_e] += yT_e[:, :n].T.

Measured (seed-0 inputs, 8 cores): relative error ~4.4e-3 vs the fp32
reference; timeline-sim ~188 us total (router ~11 us + expert ~177 us;
expert PE-busy ~171 us ~= the 1-cycle/row roofline at CA=1064).
"""

import numpy as np
import ml_dtypes

import concourse.bacc as bacc
import concourse.mybir as mybir
import concourse.tile as tile
from concourse.bass_utils import run_bass_kernel_spmd

P = 128
E = 8
H = 1024
F = 2048
T = 4096
NCORES = 8
TSH = T // NCORES  # tokens per core in router phase
HO = H // P  # 8
FO = F // P  # 16
F32 = mybir.dt.float32
BF16 = mybir.dt.bfloat16
BF_NP = ml_dtypes.bfloat16

_NC_CACHE = {}
_W_CACHE = {}


def _token_chunks(CAL):
    """Split CAL into free-dim chunks <= 512 (last chunk smallest)."""
    chunks = []
    t0 = 0
    while t0 < CAL:
        tl = min(512, CAL - t0)
        chunks.append((t0, tl))
        t0 += tl
    return chunks


def _build_router():
    nc = bacc.Bacc("TRN2", target_bir_lowering=False, debug=False,
                   enable_partition_id=False)
    xT = nc.dram_tensor("xT", [H, TSH], F32, kind="ExternalInput")
    rwT = nc.dram_tensor("rwT", [H, E], F32, kind="ExternalInput")
    c_out = nc.dram_tensor("c", [TSH, E], F32, kind="ExternalOutput")
    NT = TSH // P  # token blocks
    with tile.TileContext(nc) as tc:
        with tc.tile_pool(name="xp", bufs=1) as xp, \
             tc.tile_pool(name="wp", bufs=1) as wp, \
             tc.tile_pool(name="sp", bufs=4) as sp, \
             tc.tile_pool(name="cp", bufs=1) as cp, \
             tc.tile_pool(name="ps", bufs=4, space="PSUM") as ps:
            rw = wp.tile([P, HO, E], F32)
            # rw rides the otherwise-idle Act queue so xt0 leads on sync.
            nc.scalar.dma_start(rw[:], xT_rw_src(xT, rwT))
            xts = []
            for tt in range(NT):
                xt = xp.tile([P, HO, P], F32, tag=f"xt{tt}", name=f"xt{tt}")
                nc.sync.dma_start(
                    xt[:],
                    xT.ap()[:, tt * P:(tt + 1) * P].rearrange(
                        "(ho p) t -> p ho t", p=P))
                xts.append(xt)
            pls = [ps.tile([P, E], F32, tag="pl", name=f"pl{tt}")
                   for tt in range(NT)]
            for tt in range(NT):
                for ho in range(HO):
                    nc.tensor.matmul(pls[tt][:], xts[tt][:, ho, :],
                                     rw[:, ho, :],
                                     start=(ho == 0), stop=(ho == HO - 1))
            cgall = cp.tile([P, NT, E], F32)
            for tt in range(NT):
                z = sp.tile([P, E], F32, tag="z")
                nc.scalar.activation(z[:], pls[tt][:],
                                     mybir.ActivationFunctionType.Exp)
                m8 = sp.tile([P, 8], F32, tag="m8")
                nc.vector.max(m8[:], z[:])
                s2 = sp.tile([P, 1], F32, tag="s2")
                nc.vector.tensor_add(s2[:], m8[:, 0:1], m8[:, 1:2])
                rec = sp.tile([P, 1], F32, tag="rec")
                nc.vector.reciprocal(rec[:], s2[:])
                cm = sp.tile([P, E], F32, tag="cm")
                nc.vector.scalar_tensor_tensor(
                    cm[:], z[:], m8[:, 1:2], z[:],
                    op0=mybir.AluOpType.is_ge, op1=mybir.AluOpType.mult)
                nc.vector.tensor_scalar_mul(cgall[:, tt, :], cm[:],
                                            rec[:, 0:1])
            nc.sync.dma_start(
                c_out.ap().rearrange("(tt p) e -> p tt e", p=P), cgall[:])
    nc.compile()
    return nc


def xT_rw_src(xT, rwT):
    return rwT.ap().rearrange("(ho p) e -> p ho e", p=P)


def _build_expert(C, CA):
    CAL = min(C, ((CA + 7) // 8) * 8)
    chunks = _token_chunks(CAL)
    nchunks = len(chunks)
    nc = bacc.Bacc("TRN2", target_bir_lowering=False, debug=False,
                   enable_partition_id=False)
    xgT = nc.dram_tensor("xgT", [H, C], BF16, kind="ExternalInput")
    cb = nc.dram_tensor("cb", [P, CAL], F32, kind="ExternalInput")
    wvt = nc.dram_tensor("wvt", [FO, P, 2, HO, P], BF16, kind="ExternalInput")
    w2t = nc.dram_tensor("w2t", [FO, P, HO, P], BF16, kind="ExternalInput")
    y = nc.dram_tensor("y", [H, C], BF16, kind="ExternalOutput")
    with tile.TileContext(nc) as tc:
        with tc.tile_pool(name="xp", bufs=1) as xp, \
             tc.tile_pool(name="hp", bufs=1) as hp, \
             tc.tile_pool(name="wp", bufs=4) as wp, \
             tc.tile_pool(name="w2p", bufs=16) as w2p, \
             tc.tile_pool(name="cp", bufs=1) as cp, \
             tc.tile_pool(name="scp", bufs=2) as scp, \
             tc.tile_pool(name="yp", bufs=2) as yp, \
             tc.tile_pool(name="ps", bufs=3, space="PSUM") as ps, \
             tc.tile_pool(name="psb", bufs=2, space="PSUM") as psb:

            # --- DMA issue: critical path (xg0, wv0 first half) on sync ---
            xgs = []
            xg0 = xp.tile([P, CAL], BF16, tag="xg0", name="xg0")
            nc.sync.dma_start(xg0[:], xgT.ap()[0:P, :CAL])
            xgs.append(xg0)

            def load_wv(fo, split):
                wv = wp.tile([P, 2, HO, P], BF16, tag="wv", name=f"wv{fo}")
                if split:
                    HH = HO // 2
                    nc.sync.dma_start(wv[:, :, 0:HH, :],
                                      wvt.ap()[fo, :, :, 0:HH, :])
                    nc.sync.dma_start(wv[:, :, HH:HO, :],
                                      wvt.ap()[fo, :, :, HH:HO, :])
                else:
                    nc.sync.dma_start(wv[:], wvt.ap()[fo])
                return wv

            wv0 = load_wv(0, split=True)
            wv1 = load_wv(1, split=False)

            for ho in range(1, HO):
                xg = xp.tile([P, CAL], BF16, tag=f"xg{ho}", name=f"xg{ho}")
                nc.gpsimd.dma_start(xg[:], xgT.ap()[ho * P:(ho + 1) * P, :CAL])
                xgs.append(xg)
            cbt = cp.tile([P, CAL], F32)
            nc.gpsimd.dma_start(cbt[:], cb.ap())
            w2s = []
            for fo in range(FO):
                w2 = w2p.tile([P, HO, P], BF16, tag="w2", name=f"w2_{fo}")
                nc.gpsimd.dma_start(w2[:], w2t.ap()[fo])
                w2s.append(w2)

            h = hp.tile([P, FO, CAL], BF16)

            def glu_tail(fo, t0, tl, p1, p2):
                sc = scp.tile([P, 512], F32, tag="sc", name="sc")[:, :tl]
                nc.scalar.activation(sc, p1,
                                     mybir.ActivationFunctionType.Silu)
                nc.vector.tensor_mul(h[:, fo, t0:t0 + tl], sc, p2)

            # --- Phase A prologue: fo=0 (all chunks) + fo=1 chunk0,
            # ho-outer so the PE chases the streaming xg tiles. ---
            t00, tl0 = chunks[0]
            ps1s = [ps.tile([P, 512], F32, tag="ps1", name=f"ps1_{i}")[:, :tl]
                    for i, (t0, tl) in enumerate(chunks)]
            ps2s = [ps.tile([P, 512], F32, tag="ps2", name=f"ps2_{i}")[:, :tl]
                    for i, (t0, tl) in enumerate(chunks)]
            pre1 = psb.tile([P, 512], F32, tag="psy", name="pre1")[:, :tl0]
            pre2 = psb.tile([P, 512], F32, tag="psy", name="pre2")[:, :tl0]
            for ho in range(HO):
                st, sp_ = (ho == 0), (ho == HO - 1)
                for i, (t0, tl) in enumerate(chunks):
                    nc.tensor.matmul(ps1s[i], wv0[:, 0, ho, :],
                                     xgs[ho][:, t0:t0 + tl], start=st, stop=sp_)
                    nc.tensor.matmul(ps2s[i], wv0[:, 1, ho, :],
                                     xgs[ho][:, t0:t0 + tl], start=st, stop=sp_)
                nc.tensor.matmul(pre1, wv1[:, 0, ho, :],
                                 xgs[ho][:, t00:t00 + tl0], start=st, stop=sp_)
                nc.tensor.matmul(pre2, wv1[:, 1, ho, :],
                                 xgs[ho][:, t00:t00 + tl0], start=st, stop=sp_)
            for i, (t0, tl) in enumerate(chunks):
                glu_tail(0, t0, tl, ps1s[i], ps2s[i])
            glu_tail(1, t00, tl0, pre1, pre2)

            # --- Phase A steady state ---
            for fo in range(1, FO):
                if fo == 1:
                    wv = wv1
                    fo_chunks = chunks[1:]
                else:
                    wv = load_wv(fo, split=False)
                    fo_chunks = chunks
                for i, (t0, tl) in enumerate(fo_chunks):
                    p1 = ps.tile([P, 512], F32, tag="ps1", name="p1")[:, :tl]
                    p2 = ps.tile([P, 512], F32, tag="ps2", name="p2")[:, :tl]
                    for ho in range(HO):
                        st, sp_ = (ho == 0), (ho == HO - 1)
                        nc.tensor.matmul(p1, wv[:, 0, ho, :],
                                         xgs[ho][:, t0:t0 + tl],
                                         start=st, stop=sp_)
                        nc.tensor.matmul(p2, wv[:, 1, ho, :],
                                         xgs[ho][:, t0:t0 + tl],
                                         start=st, stop=sp_)
                    glu_tail(fo, t0, tl, p1, p2)

            # --- Phase B: yT[hb] = (sum_fo w2[fo,hb]^T h[fo]) * cb ---
            for hb in range(HO):
                yt = yp.tile([P, CAL], BF16, tag="yt", name=f"yt{hb}")
                for (t0, tl) in chunks:
                    psy = psb.tile([P, 512], F32, tag="psy", name="psy")[:, :tl]
                    for fo in range(FO):
                        nc.tensor.matmul(psy, w2s[fo][:, hb, :],
                                         h[:, fo, t0:t0 + tl],
                                         start=(fo == 0), stop=(fo == FO - 1))
                    nc.vector.tensor_mul(yt[:, t0:t0 + tl], psy,
                                         cbt[:, t0:t0 + tl])
                nc.scalar.dma_start(y.ap()[hb * P:(hb + 1) * P, 0:CAL], yt[:])
    nc.compile()
    return nc


def _get_nc(key, builder):
    if key not in _NC_CACHE:
        _NC_CACHE[key] = builder()
    return _NC_CACHE[key]


def _tile_weights(w1, v1, w2):
    """Pre-tile expert weights (bf16) for large-descriptor DMA.

    wvt:  [E, FO, 128(h), 2, HO, 128(f)]  (w1/v1 lhsT tiles, interleaved)
    w2bt: [E, FO, 128(f), HO, 128(h)]     (lhsT tiles of the [F, H] mats)
    """
    key = (w1.shape, w1.dtype.str, w1[0, 0, :4].tobytes(), w2[0, 0, :4].tobytes(),
           v1[0, 0, :4].tobytes(), float(w1[-1, -1, -1]), float(w2[-1, -1, -1]))
    if key in _W_CACHE:
        return _W_CACHE[key]
    # w1[e] is [F, H]; lhsT tile (fo): [p_h, ho, q_f] = w1[e][fo*128+q, ho*128+p]
    w1t = w1.reshape(E, FO, P, HO, P).transpose(0, 1, 4, 3, 2)
    v1t = v1.reshape(E, FO, P, HO, P).transpose(0, 1, 4, 3, 2)
    wvt = np.ascontiguousarray(
        np.stack([w1t, v1t], axis=3).astype(BF_NP))  # [E,FO,P,2,HO,P]
    # w2[e] is [F, H]; lhsT tile (fo, hb): [p_f, j_h] = w2[e][fo*128+p, hb*128+j]
    w2bt = np.ascontiguousarray(w2.reshape(E, FO, P, HO, P).astype(BF_NP))
    _W_CACHE.clear()
    _W_CACHE[key] = (wvt, w2bt)
    return wvt, w2bt


def kernel(x, router_w, w1, v1, w2):
    x = np.asarray(x, dtype=np.float32)
    router_w = np.asarray(router_w, dtype=np.float32)
    w1 = np.asarray(w1, dtype=np.float32)
    v1 = np.asarray(v1, dtype=np.float32)
    w2 = np.asarray(w2, dtype=np.float32)

    xf = x.reshape(T, H)
    xT = np.ascontiguousarray(xf.T)  # [H, T] fp32 (router)
    xT16 = xT.astype(BF_NP)          # [H, T] bf16 (expert gather)
    rwT = np.ascontiguousarray(router_w.T)  # [H, E]

    # ---- Phase 1: router on device (data-parallel over tokens) ----
    nc1 = _get_nc("router", _build_router)
    in1 = [{"xT": np.ascontiguousarray(xT[:, i * TSH:(i + 1) * TSH]),
            "rwT": rwT}
           for i in range(NCORES)]
    r1 = run_bass_kernel_spmd(nc1, in1, core_ids=list(range(NCORES)))
    c = np.concatenate([r["c"] for r in r1.results], axis=0)  # [T, E]

    # ---- Host dispatch: gather tokens per expert (data movement only) ----
    idxs = [np.flatnonzero(c[:, e] != 0.0) for e in range(E)]
    maxc = max(len(ix) for ix in idxs)
    # Per-launch capacity; >1280 tokens per expert (never happens with
    # balanced routing) is handled by running the same NEFF multiple times.
    C = max(1152, min(1280, ((maxc + 127) // 128) * 128))
    nseg = (maxc + C - 1) // C

    wvt, w2bt = _tile_weights(w1, v1, w2)

    out = np.zeros((T, H), np.float32)
    for seg in range(nseg):
        segixs = [idxs[e][seg * C:(seg + 1) * C] for e in range(E)]
        CA = max(1, max(len(ix) for ix in segixs))  # exact active count
        CAL = min(C, ((CA + 7) // 8) * 8)
        nc2 = _get_nc(("expert", C, CAL), lambda: _build_expert(C, CAL))
        in2 = []
        for e in range(E):
            ix = segixs[e]
            xgT = np.zeros((H, C), BF_NP)
            xgT[:, :len(ix)] = xT16[:, ix]
            cge = np.zeros((CAL,), np.float32)
            cge[:len(ix)] = c[ix, e]
            cb = np.ascontiguousarray(np.broadcast_to(cge, (P, CAL)))
            in2.append({"xgT": xgT, "cb": cb,
                        "wvt": wvt[e], "w2t": w2bt[e]})
        r2 = run_bass_kernel_spmd(nc2, in2, core_ids=list(range(NCORES)))
        # ---- Host combine: scatter-add per-expert outputs ----
        for e in range(E):
            ix = segixs[e]
            yT = r2.results[e]["y"]  # [H, C] bf16
            out[ix] += yT[:, :len(ix)].T.astype(np.float32)
    return out.reshape(x.shape)


# revision 7
# speedup vs baseline: 1.0563x; 1.0470x over previous
"""MoE FFN (8 experts, top-2, GLU) on 8 Trainium2 NeuronCores.

Strategy
--------
Phase 1 (on-device, data-parallel over tokens): each core computes router
logits in fp32 for its 512-token shard, then top-2 gate weights
c[t, e] = z_e / (z_1 + z_2) with z = exp(logit) for the two largest
(identical to softmax + top-k + L1-normalize; no max-subtraction is
needed because |logit| <~ 5 for these scales).  Router stays fp32 so the
top-2 SELECTION matches the fp32 reference exactly (bf16 logits would
flip near-tie tokens and blow the error budget).

Host dispatch (data movement only): for each expert, gather the columns of
bf16 x^T for its routed tokens into a fixed-capacity buffer.

Phase 2 (on-device, expert-parallel, bf16 operands / fp32 accumulate):
core e computes the GLU FFN of expert e over its CA gathered tokens:
    h = silu(w1t^T xg) * (v1t^T xg)      [F, CA]   (h stored bf16)
    yT[hb] = sum_fo w2t[fo,hb]^T h[fo]   [H, CA]   (output transposed:
             H on partitions, tokens on the free dim -> no token-padding
             to 128 in the second GEMM)
    yT *= cb                             (gate broadcast along partitions)
bf16 matmuls run at the same 1 cycle/row as fp32r but halve every DMA
byte, which shrinks the pipeline head and keeps the PE continuously fed.
w1/v1 ship interleaved as one DMA per fo-block; x chunks go out on the
gpsimd (SWDGE) queue so the SP sequencer only carries the critical path.

Host combine (data movement only): out[idx_e] += yT_e[:, :n].T.

Measured (seed-0 inputs, 8 cores): relative error ~4.4e-3 vs the fp32
reference; timeline-sim ~188 us total (router ~11 us + expert ~177 us;
expert PE-busy ~171 us ~= the 1-cycle/row roofline at CA=1064).
"""

import numpy as np
import ml_dtypes

import concourse.bacc as bacc
import concourse.mybir as mybir
import concourse.tile as tile
from concourse.bass_utils import run_bass_kernel_spmd

P = 128
E = 8
H = 1024
F = 2048
T = 4096
NCORES = 8
TSH = T // NCORES  # tokens per core in router phase
HO = H // P  # 8
FO = F // P  # 16
F32 = mybir.dt.float32
BF16 = mybir.dt.bfloat16
BF_NP = ml_dtypes.bfloat16

_NC_CACHE = {}
_W_CACHE = {}


def _token_chunks(CAL):
    """Split CAL into free-dim chunks <= 512 (last chunk smallest)."""
    chunks = []
    t0 = 0
    while t0 < CAL:
        tl = min(512, CAL - t0)
        chunks.append((t0, tl))
        t0 += tl
    return chunks


def _build_router():
    nc = bacc.Bacc("TRN2", target_bir_lowering=False, debug=False,
                   enable_partition_id=False)
    xT = nc.dram_tensor("xT", [H, TSH], F32, kind="ExternalInput")
    rwT = nc.dram_tensor("rwT", [H, E], F32, kind="ExternalInput")
    c_out = nc.dram_tensor("c", [TSH, E], F32, kind="ExternalOutput")
    NT = TSH // P  # token blocks
    with tile.TileContext(nc) as tc:
        with tc.tile_pool(name="xp", bufs=1) as xp, \
             tc.tile_pool(name="wp", bufs=1) as wp, \
             tc.tile_pool(name="sp", bufs=4) as sp, \
             tc.tile_pool(name="cp", bufs=1) as cp, \
             tc.tile_pool(name="ps", bufs=4, space="PSUM") as ps:
            rw = wp.tile([P, HO, E], F32)
            # rw rides the otherwise-idle Act queue so xt0 leads on sync.
            nc.scalar.dma_start(rw[:], xT_rw_src(xT, rwT))
            xts = []
            for tt in range(NT):
                xt = xp.tile([P, HO, P], F32, tag=f"xt{tt}", name=f"xt{tt}")
                nc.sync.dma_start(
                    xt[:],
                    xT.ap()[:, tt * P:(tt + 1) * P].rearrange(
                        "(ho p) t -> p ho t", p=P))
                xts.append(xt)
            pls = [ps.tile([P, E], F32, tag="pl", name=f"pl{tt}")
                   for tt in range(NT)]
            for tt in range(NT):
                for ho in range(HO):
                    nc.tensor.matmul(pls[tt][:], xts[tt][:, ho, :],
                                     rw[:, ho, :],
                                     start=(ho == 0), stop=(ho == HO - 1))
            cgall = cp.tile([P, NT, E], F32)
            for tt in range(NT):
                z = sp.tile([P, E], F32, tag="z")
                nc.scalar.activation(z[:], pls[tt][:],
                                     mybir.ActivationFunctionType.Exp)
                m8 = sp.tile([P, 8], F32, tag="m8")
                nc.vector.max(m8[:], z[:])
                s2 = sp.tile([P, 1], F32, tag="s2")
                nc.vector.tensor_add(s2[:], m8[:, 0:1], m8[:, 1:2])
                rec = sp.tile([P, 1], F32, tag="rec")
                nc.vector.reciprocal(rec[:], s2[:])
                cm = sp.tile([P, E], F32, tag="cm")
                nc.vector.scalar_tensor_tensor(
                    cm[:], z[:], m8[:, 1:2], z[:],
                    op0=mybir.AluOpType.is_ge, op1=mybir.AluOpType.mult)
                nc.vector.tensor_scalar_mul(cgall[:, tt, :], cm[:],
                                            rec[:, 0:1])
            nc.sync.dma_start(
                c_out.ap().rearrange("(tt p) e -> p tt e", p=P), cgall[:])
    nc.compile()
    return nc


def xT_rw_src(xT, rwT):
    return rwT.ap().rearrange("(ho p) e -> p ho e", p=P)


def _build_expert(C, CA):
    CAL = min(C, ((CA + 7) // 8) * 8)
    chunks = _token_chunks(CAL)
    nchunks = len(chunks)
    nc = bacc.Bacc("TRN2", target_bir_lowering=False, debug=False,
                   enable_partition_id=False)
    xgT = nc.dram_tensor("xgT", [H, C], BF16, kind="ExternalInput")
    cb = nc.dram_tensor("cb", [P, CAL], F32, kind="ExternalInput")
    wvt = nc.dram_tensor("wvt", [FO, P, 2, HO, P], BF16, kind="ExternalInput")
    w2t = nc.dram_tensor("w2t", [FO, P, HO, P], BF16, kind="ExternalInput")
    y = nc.dram_tensor("y", [H, C], BF16, kind="ExternalOutput")
    with tile.TileContext(nc) as tc:
        with tc.tile_pool(name="xp", bufs=1) as xp, \
             tc.tile_pool(name="hp", bufs=1) as hp, \
             tc.tile_pool(name="wp", bufs=4) as wp, \
             tc.tile_pool(name="w2p", bufs=16) as w2p, \
             tc.tile_pool(name="cp", bufs=1) as cp, \
             tc.tile_pool(name="scp", bufs=2) as scp, \
             tc.tile_pool(name="yp", bufs=2) as yp, \
             tc.tile_pool(name="ps", bufs=3, space="PSUM") as ps, \
             tc.tile_pool(name="psb", bufs=2, space="PSUM") as psb:

            # --- DMA issue.  Tile dep granularity is per-tile, so the
            # pieces the first matmuls need are SEPARATE tiles: xg(ho=0)
            # splits into [0:512] + [512:CAL] on the Act queue, fo=0's
            # weights into two half tiles on sync. ---
            HH = HO // 2
            c0w = min(512, CAL)
            xg0a = xp.tile([P, c0w], BF16, tag="xg0a", name="xg0a")
            nc.scalar.dma_start(xg0a[:], xgT.ap()[0:P, 0:c0w])
            xg0b = None
            if CAL > 512:
                xg0b = xp.tile([P, CAL - 512], BF16, tag="xg0b", name="xg0b")
                nc.scalar.dma_start(xg0b[:], xgT.ap()[0:P, 512:CAL])

            def xg_slice(ho, t0, tl):
                if ho == 0:
                    if t0 < 512:
                        return xg0a[:, t0:t0 + tl]
                    return xg0b[:, t0 - 512:t0 - 512 + tl]
                return xgs[ho][:, t0:t0 + tl]

            wv0a = wp.tile([P, 2, HH, P], BF16, tag="wv0a", name="wv0a")
            nc.sync.dma_start(wv0a[:], wvt.ap()[0, :, :, 0:HH, :])
            wv0b = wp.tile([P, 2, HH, P], BF16, tag="wv0b", name="wv0b")
            nc.sync.dma_start(wv0b[:], wvt.ap()[0, :, :, HH:HO, :])

            def wv0_slice(m, ho):
                t = wv0a if ho < HH else wv0b
                return t[:, m, ho % HH, :]

            def load_wv(fo):
                wv = wp.tile([P, 2, HO, P], BF16, tag="wv", name=f"wv{fo}")
                nc.sync.dma_start(wv[:], wvt.ap()[fo])
                return wv

            wv1 = load_wv(1)

            xgs = [None]
            for ho in range(1, HO):
                xg = xp.tile([P, CAL], BF16, tag=f"xg{ho}", name=f"xg{ho}")
                nc.gpsimd.dma_start(xg[:], xgT.ap()[ho * P:(ho + 1) * P, :CAL])
                xgs.append(xg)
            cbt = cp.tile([P, CAL], F32)
            nc.gpsimd.dma_start(cbt[:], cb.ap())
            w2s = []
            for fo in range(FO):
                w2 = w2p.tile([P, HO, P], BF16, tag="w2", name=f"w2_{fo}")
                nc.gpsimd.dma_start(w2[:], w2t.ap()[fo])
                w2s.append(w2)

            h = hp.tile([P, FO, CAL], BF16)

            def glu_tail(fo, t0, tl, p1, p2):
                sc = scp.tile([P, 512], F32, tag="sc", name="sc")[:, :tl]
                nc.scalar.activation(sc, p1,
                                     mybir.ActivationFunctionType.Silu)
                nc.vector.tensor_mul(h[:, fo, t0:t0 + tl], sc, p2)

            # --- Phase A prologue: fo=0 (all chunks) + fo=1 chunk0,
            # ho-outer so the PE chases the streaming xg tiles. ---
            t00, tl0 = chunks[0]
            ps1s = [ps.tile([P, 512], F32, tag="ps1", name=f"ps1_{i}")[:, :tl]
                    for i, (t0, tl) in enumerate(chunks)]
            ps2s = [ps.tile([P, 512], F32, tag="ps2", name=f"ps2_{i}")[:, :tl]
                    for i, (t0, tl) in enumerate(chunks)]
            pre1 = psb.tile([P, 512], F32, tag="psy", name="pre1")[:, :tl0]
            pre2 = psb.tile([P, 512], F32, tag="psy", name="pre2")[:, :tl0]
            for ho in range(HO):
                st, sp_ = (ho == 0), (ho == HO - 1)
                for i, (t0, tl) in enumerate(chunks):
                    nc.tensor.matmul(ps1s[i], wv0_slice(0, ho),
                                     xg_slice(ho, t0, tl), start=st, stop=sp_)
                    nc.tensor.matmul(ps2s[i], wv0_slice(1, ho),
                                     xg_slice(ho, t0, tl), start=st, stop=sp_)
                nc.tensor.matmul(pre1, wv1[:, 0, ho, :],
                                 xg_slice(ho, t00, tl0), start=st, stop=sp_)
                nc.tensor.matmul(pre2, wv1[:, 1, ho, :],
                                 xg_slice(ho, t00, tl0), start=st, stop=sp_)
            for i, (t0, tl) in enumerate(chunks):
                glu_tail(0, t0, tl, ps1s[i], ps2s[i])
            glu_tail(1, t00, tl0, pre1, pre2)

            # --- Phase A steady state ---
            for fo in range(1, FO):
                if fo == 1:
                    wv = wv1
                    fo_chunks = chunks[1:]
                else:
                    wv = load_wv(fo)
                    fo_chunks = chunks
                for i, (t0, tl) in enumerate(fo_chunks):
                    p1 = ps.tile([P, 512], F32, tag="ps1", name="p1")[:, :tl]
                    p2 = ps.tile([P, 512], F32, tag="ps2", name="p2")[:, :tl]
                    for ho in range(HO):
                        st, sp_ = (ho == 0), (ho == HO - 1)
                        nc.tensor.matmul(p1, wv[:, 0, ho, :],
                                         xg_slice(ho, t0, tl),
                                         start=st, stop=sp_)
                        nc.tensor.matmul(p2, wv[:, 1, ho, :],
                                         xg_slice(ho, t0, tl),
                                         start=st, stop=sp_)
                    glu_tail(fo, t0, tl, p1, p2)

            # --- Phase B: yT[hb] = (sum_fo w2[fo,hb]^T h[fo]) * cb ---
            # psy rotates across all three psum pools (8 banks total) so
            # the gate-mul never stalls the next accumulation group; the
            # last hb stores per-chunk so only a tiny store trails the
            # final matmul.
            pool_cycle = [(psb, "psy"), (ps, "ps1"), (ps, "ps2")]
            gi = 0
            for hb in range(HO):
                yt = yp.tile([P, CAL], BF16, tag="yt", name=f"yt{hb}")
                last_hb = (hb == HO - 1)
                for ci, (t0, tl) in enumerate(chunks):
                    pool, ptag = pool_cycle[gi % 3]
                    gi += 1
                    psy = pool.tile([P, 512], F32, tag=ptag,
                                    name="psy")[:, :tl]
                    for fo in range(FO):
                        nc.tensor.matmul(psy, w2s[fo][:, hb, :],
                                         h[:, fo, t0:t0 + tl],
                                         start=(fo == 0), stop=(fo == FO - 1))
                    nc.vector.tensor_mul(yt[:, t0:t0 + tl], psy,
                                         cbt[:, t0:t0 + tl])
                    if last_hb:
                        nc.sync.dma_start(
                            y.ap()[hb * P:(hb + 1) * P, t0:t0 + tl],
                            yt[:, t0:t0 + tl])
                if not last_hb:
                    nc.sync.dma_start(y.ap()[hb * P:(hb + 1) * P, 0:CAL],
                                      yt[:])
    nc.compile()
    return nc


def _get_nc(key, builder):
    if key not in _NC_CACHE:
        _NC_CACHE[key] = builder()
    return _NC_CACHE[key]


def _tile_weights(w1, v1, w2):
    """Pre-tile expert weights (bf16) for large-descriptor DMA.

    wvt:  [E, FO, 128(h), 2, HO, 128(f)]  (w1/v1 lhsT tiles, interleaved)
    w2bt: [E, FO, 128(f), HO, 128(h)]     (lhsT tiles of the [F, H] mats)
    """
    key = (w1.shape, w1.dtype.str, w1[0, 0, :4].tobytes(), w2[0, 0, :4].tobytes(),
           v1[0, 0, :4].tobytes(), float(w1[-1, -1, -1]), float(w2[-1, -1, -1]))
    if key in _W_CACHE:
        return _W_CACHE[key]
    # w1[e] is [F, H]; lhsT tile (fo): [p_h, ho, q_f] = w1[e][fo*128+q, ho*128+p]
    w1t = w1.reshape(E, FO, P, HO, P).transpose(0, 1, 4, 3, 2)
    v1t = v1.reshape(E, FO, P, HO, P).transpose(0, 1, 4, 3, 2)
    wvt = np.ascontiguousarray(
        np.stack([w1t, v1t], axis=3).astype(BF_NP))  # [E,FO,P,2,HO,P]
    # w2[e] is [F, H]; lhsT tile (fo, hb): [p_f, j_h] = w2[e][fo*128+p, hb*128+j]
    w2bt = np.ascontiguousarray(w2.reshape(E, FO, P, HO, P).astype(BF_NP))
    _W_CACHE.clear()
    _W_CACHE[key] = (wvt, w2bt)
    return wvt, w2bt


def kernel(x, router_w, w1, v1, w2):
    x = np.asarray(x, dtype=np.float32)
    router_w = np.asarray(router_w, dtype=np.float32)
    w1 = np.asarray(w1, dtype=np.float32)
    v1 = np.asarray(v1, dtype=np.float32)
    w2 = np.asarray(w2, dtype=np.float32)

    xf = x.reshape(T, H)
    xT = np.ascontiguousarray(xf.T)  # [H, T] fp32 (router)
    xT16 = xT.astype(BF_NP)          # [H, T] bf16 (expert gather)
    rwT = np.ascontiguousarray(router_w.T)  # [H, E]

    # ---- Phase 1: router on device (data-parallel over tokens) ----
    nc1 = _get_nc("router", _build_router)
    in1 = [{"xT": np.ascontiguousarray(xT[:, i * TSH:(i + 1) * TSH]),
            "rwT": rwT}
           for i in range(NCORES)]
    r1 = run_bass_kernel_spmd(nc1, in1, core_ids=list(range(NCORES)))
    c = np.concatenate([r["c"] for r in r1.results], axis=0)  # [T, E]

    # ---- Host dispatch: gather tokens per expert (data movement only) ----
    idxs = [np.flatnonzero(c[:, e] != 0.0) for e in range(E)]
    maxc = max(len(ix) for ix in idxs)
    # Per-launch capacity; >1280 tokens per expert (never happens with
    # balanced routing) is handled by running the same NEFF multiple times.
    C = max(1152, min(1280, ((maxc + 127) // 128) * 128))
    nseg = (maxc + C - 1) // C

    wvt, w2bt = _tile_weights(w1, v1, w2)

    out = np.zeros((T, H), np.float32)
    for seg in range(nseg):
        segixs = [idxs[e][seg * C:(seg + 1) * C] for e in range(E)]
        CA = max(1, max(len(ix) for ix in segixs))  # exact active count
        CAL = min(C, ((CA + 7) // 8) * 8)
        nc2 = _get_nc(("expert", C, CAL), lambda: _build_expert(C, CAL))
        in2 = []
        for e in range(E):
            ix = segixs[e]
            xgT = np.zeros((H, C), BF_NP)
            xgT[:, :len(ix)] = xT16[:, ix]
            cge = np.zeros((CAL,), np.float32)
            cge[:len(ix)] = c[ix, e]
            cb = np.ascontiguousarray(np.broadcast_to(cge, (P, CAL)))
            in2.append({"xgT": xgT, "cb": cb,
                        "wvt": wvt[e], "w2t": w2bt[e]})
        r2 = run_bass_kernel_spmd(nc2, in2, core_ids=list(range(NCORES)))
        # ---- Host combine: scatter-add per-expert outputs ----
        for e in range(E):
            ix = segixs[e]
            yT = r2.results[e]["y"]  # [H, C] bf16
            out[ix] += yT[:, :len(ix)].T.astype(np.float32)
    return out.reshape(x.shape)
